# revision 1
# baseline (speedup 1.0000x reference)
"""Causal self-attention Trainium2 kernel (Bass/Tile), 8-core SPMD.

Problem: nn_CausalSelfAttention (B=2, T=2048, C=768, H=8 heads, D=96).

Sharding: core = b*4 + hg with b in {0,1} batches and hg in {0..3} head-groups.
Each core computes attention for ONE batch and TWO heads, plus that head-pair's
slice of the output projection. Host sums the 4 per-batch partials (bf16
partials, f32 accumulation).

All matmul inputs are bf16 (1 cycle/row on the PE, same as f32r at >=256
moving but with no short-matmul penalty); PSUM accumulates f32.

Key structural choices vs the f32r baseline:
  1. x^T is prepared on the HOST (free) and DMA'd as [128, qr, cc, 512]
     bf16 - no PE transposes at all.
  2. Q^T,K^T = Wqk^T @ x^T in 3 exact 128-feature groups (Q pre-scaled by
     1/sqrt(D) folded into W/b on host); V is computed NATURAL
     ([t, d], moving dim 192) so V_aug needs no transposes either. V's bias
     is folded into the output-projection bias row on host
     (softmax rows sum to 1 => (P(V+1 b_v^T))/l = PV/l + b_v, and
     b_v @ W_proj is a constant row added to b_proj).
  3. Scores computed transposed S^T[k,q] = K^T_blk.T @ Q^T, exp on ACT
     (bf16 out), causality via block trimming + one affine_select triangle
     per diagonal block (scores bounded, no max-subtraction).
  4. y_aug^T accumulated in PSUM with a ones-column in V_aug giving the
     softmax denominator l; normalization = DVE reciprocal + GPSIMD
     partition_broadcast + DVE multiply.
  5. out = sum_h yn_h^T.T @ W_aug_h, W_aug row 96 carries the combined bias
     exactly once across the 8-core sum. Superblocks processed in order
     1,2,3,0 with U (output-projection) matmuls interleaved as PE filler;
     the final superblock's normalization+projection is chunked per t-block
     to shorten the tail.
"""
import sys

sys.path.insert(0, "/opt/trn_rl_repo")

import numpy as np
import ml_dtypes

import concourse.bacc as bacc
import concourse.mybir as mybir
import concourse.tile as tile
from concourse.bass_utils import run_bass_kernel_spmd

F32 = mybir.dt.float32
BF16 = mybir.dt.bfloat16
NPBF = ml_dtypes.bfloat16

B, T, C = 2, 2048, 768
H, D = 8, 96
NB = T // 128            # 16 t-blocks of 128
NSUP = T // 512          # 4 q-superblocks of 512
CC = C // 128            # 6 c-chunks
SCALE = 1.0 / np.sqrt(D)

# QK PSUM->SBUF evacuation pieces: (group, r0, j, d0, w) with the 384
# packed features split at head boundaries (96) and partition-base
# legality limits (base 0: <=128, base 64: <=64, base 32/96: <=32).
QK_PIECES = [
    (0, 0, 0, 0, 96),
    (0, 96, 1, 0, 32),
    (1, 0, 1, 32, 32),
    (1, 32, 1, 64, 32),
    (1, 64, 2, 0, 64),
    (2, 0, 2, 64, 32),
    (2, 32, 3, 0, 32),
    (2, 64, 3, 32, 32),
    (2, 96, 3, 64, 32),
]

_NC_CACHE = None
TRACE = False          # set True (e.g. from test.py) to capture an NTFF profile
LAST_RESULT = None     # BassKernelResults of the most recent run
MM_LABELS = []         # program-order labels of tensor-engine matmuls
FOLLOW = False         # debug: log tile-inserted deps for qr1 QK evacs


def _build():
    nc = bacc.Bacc(None, target_bir_lowering=False)

    MM_LABELS.clear()
    _raw_matmul = nc.tensor.matmul

    def _mm(out, lhsT, rhs, _label="?", **kw):
        MM_LABELS.append(_label)
        return _raw_matmul(out, lhsT, rhs, **kw)

    xT_d = nc.declare_dram_parameter("xT", [128, NSUP, CC, 512], BF16, isOutput=False)
    wqkv_d = nc.declare_dram_parameter("wqkv", [128, CC, 6 * D], BF16, isOutput=False)
    bqk_d = nc.declare_dram_parameter("bqk", [128, 3], F32, isOutput=False)
    wga_d = nc.declare_dram_parameter("wga", [D + 1, 2, C], BF16, isOutput=False)
    out_d = nc.declare_dram_parameter("out", [T, C], BF16, isOutput=True)

    Exp = mybir.ActivationFunctionType.Exp
    Ident = mybir.ActivationFunctionType.Identity

    with tile.TileContext(nc) as tc:
        with tc.sbuf_pool(name="persist", bufs=1) as persist:
            bqk = persist.tile([128, 3], F32, tag="bqk")
            wqa = persist.tile([128, CC, 6 * D], BF16, tag="wqa")
            wga = persist.tile([D + 1, 2, C], BF16, tag="wga")

            # persistent activation tensors, tiled so each is written by
            # few instructions (dependency tracking is tile-granular)
            qkT = [[persist.tile([D, 512], BF16, name=f"qkT{j}_{qr}", tag=f"qkT{j}_{qr}")
                    for qr in range(NSUP)] for j in range(4)]
            vaug = [persist.tile([128, 2, D + 1], BF16, name=f"vaug{blk}", tag=f"vaug{blk}")
                    for blk in range(NB)]
            yns = [[persist.tile([D + 1, 512], BF16, name=f"yn{si}_{h}", tag=f"yn{si}_{h}")
                    for h in range(2)] for si in range(NSUP)]

            # ---------------- Phase A: QKV projections -------------------
            # PSUM tiles must each have exactly ONE consumer instruction:
            # the Tile framework treats PSUM reads as writes, so multiple
            # readers of one PSUM tile serialize pairwise across engines.
            with (
                tc.sbuf_pool(name="xqp", bufs=3) as xqp,
                tc.psum_pool(name="psA", bufs=2) as psA,
                tc.psum_pool(name="psV", bufs=2) as psV,
            ):
                for blk in range(NB):
                    nc.vector.memset(vaug[blk][:, :, D:D + 1], 1.0)

                for qr in range(NSUP):
                    xq = xqp.tile([128, CC, 512], BF16, tag="xq", name=f"xq{qr}")
                    if qr == 0:
                        # interleave weight/x pieces so the cc-outer matmul
                        # order can start as soon as the first pieces land
                        nc.sync.dma_start(out=wqa[:, 0:1, :], in_=wqkv_d[:, 0:1, :])
                        nc.sync.dma_start(out=xq[:, 0:1, :], in_=xT_d[:, 0, 0:1, :])
                        nc.sync.dma_start(out=wqa[:, 1:3, :], in_=wqkv_d[:, 1:3, :])
                        nc.sync.dma_start(out=xq[:, 1:3, :], in_=xT_d[:, 0, 1:3, :])
                        nc.sync.dma_start(out=bqk, in_=bqk_d[:, :])
                        nc.sync.dma_start(out=wqa[:, 3:5, :], in_=wqkv_d[:, 3:5, :])
                        nc.sync.dma_start(out=xq[:, 3:5, :], in_=xT_d[:, 0, 3:5, :])
                        nc.sync.dma_start(out=wqa[:, 5:6, :], in_=wqkv_d[:, 5:6, :])
                        nc.sync.dma_start(out=xq[:, 5:6, :], in_=xT_d[:, 0, 5:6, :])
                    else:
                        nc.sync.dma_start(out=xq, in_=xT_d[:, qr])
                        if qr == 1:
                            nc.sync.dma_start(out=wga, in_=wga_d.ap())

                    # Q^T,K^T: 3 exact groups of 128 features in separate
                    # 1-bank PSUM tiles. qr0 runs cc-outer so the first
                    # matmul only needs the first w/x DMA pieces; later qr
                    # run group-outer so each group's evacuation overlaps
                    # the next group's matmuls instead of bursting.
                    # Evacuation pieces reading the same group tile
                    # serialize (PSUM reads count as writes), but chains
                    # are <=4 and parallel across groups.
                    pqg = [psA.tile([128, 512], F32, tag=f"pq{g}", name=f"pq{g}_{qr}")
                           for g in range(3)]

                    def qk_mm(g, cc):
                        _mm(
                            pqg[g],
                            wqa[:, cc, g * 128:(g + 1) * 128],
                            xq[:, cc, :],
                            start=(cc == 0), stop=(cc == CC - 1),
                            _label=f"QK.qr{qr}.cc{cc}.g{g}",
                        )

                    def qk_evac(g):
                        for i, (gg, r0, j, d0, w) in enumerate(QK_PIECES):
                            if gg != g:
                                continue
                            dst = qkT[j][qr][d0:d0 + w, :]
                            src = pqg[g][r0:r0 + w, :]
                            bias = bqk[r0:r0 + w, g:g + 1]
                            if "ADADAADDA"[i] == "A":
                                nc.scalar.activation(dst, src, Ident, bias=bias)
                            else:
                                nc.vector.tensor_scalar_add(dst, src, bias)

                    if qr == 0:
                        for cc in range(CC):
                            for g in range(3):
                                qk_mm(g, cc)
                        for g in range(3):
                            qk_evac(g)
                    else:
                        for g in ((2, 0, 1) if qr == 3 else (0, 1, 2)):
                            for cc in range(CC):
                                qk_mm(g, cc)
                            qk_evac(g)

                    # V natural: [128t, 192] per t-block, no bias (folded
                    # into wga row 96 on host); one copy per PSUM tile
                    for tb in range(4):
                        blk = qr * 4 + tb
                        pv = psV.tile([128, 2 * D], F32, tag="pv", name=f"pv{blk}")
                        for cc in range(CC):
                            _mm(
                                pv,
                                xq[:, cc, tb * 128:(tb + 1) * 128],
                                wqa[:, cc, 4 * D:6 * D],
                                start=(cc == 0), stop=(cc == CC - 1),
                                _label=f"V.qr{qr}.tb{tb}.cc{cc}",
                            )
                        nc.vector.tensor_copy(
                            vaug[blk][:, :, 0:D],
                            pv.rearrange("p (h d) -> p h d", h=2))

            # ------------ Phase B: attention + fused output projection -----
            with (
                tc.psum_pool(name="psS", bufs=2) as psS,
                tc.psum_pool(name="psY", bufs=1) as psY,
                tc.psum_pool(name="psU", bufs=1) as psU,
                tc.sbuf_pool(name="sbP", bufs=10) as sbP,
                tc.sbuf_pool(name="sbR", bufs=6) as sbR,
                tc.sbuf_pool(name="sbN", bufs=2) as sbN,
                tc.sbuf_pool(name="sbU", bufs=6) as sbU,
            ):
                def emit_S(si, kjs, h):
                    """QK^T block(s) for one head + exp (+ causal triangle).

                    kjs is one kj (diagonal-superblock, trimmed to [c0:512]) or
                    a pair of full kjs sharing one exp call. Returns list of
                    (kj, P-view)."""
                    if len(kjs) == 2:
                        ps = psS.tile([128, 1024], F32, tag="S",
                                      name=f"S{si}_{kjs[0]}p_{h}")
                        pt = sbP.tile([128, 1024], BF16, tag="P",
                                      name=f"P{si}_{kjs[0]}p_{h}")
                        for i, kj in enumerate(kjs):
                            _mm(
                                ps[:, i * 512:(i + 1) * 512],
                                qkT[2 + h][kj // 4][:, (kj % 4) * 128:(kj % 4 + 1) * 128],
                                qkT[h][si],
                                start=True, stop=True,
                                _label=f"S.si{si}.kj{kj}.h{h}",
                            )
                        nc.scalar.activation(pt, ps, Exp)
                        return [(kjs[0], pt[:, 0:512]), (kjs[1], pt[:, 512:1024])]
                    # diagonal block: both heads share one S tile and one exp
                    # call over the two valid [c0:512] ranges (strided AP)
                    kj = kjs[0]
                    m = kj - 4 * si
                    c0 = m * 128
                    ps = psS.tile([128, 1024], F32, tag="S", name=f"S{si}_{kj}_{h}")
                    for hh in range(2):
                        _mm(
                            ps[:, hh * 512 + c0:(hh + 1) * 512],
                            qkT[2 + hh][si][:, m * 128:(m + 1) * 128],  # K^T slice
                            qkT[hh][si][:, c0:512],                     # Q^T slice
                            start=True, stop=True,
                            _label=f"Sd.si{si}.kj{kj}.h{hh}",
                        )
                    pt = sbP.tile([128, 1024], BF16, tag="P", name=f"P{si}_{kj}_{h}")
                    nc.scalar.activation(
                        pt.rearrange("p (hh q) -> p hh q", hh=2)[:, :, c0:512],
                        ps.rearrange("p (hh q) -> p hh q", hh=2)[:, :, c0:512],
                        Exp,
                    )
                    for hh in range(2):
                        nc.gpsimd.affine_select(
                            out=pt[:, hh * 512 + m * 128:hh * 512 + (m + 1) * 128],
                            in_=pt[:, hh * 512 + m * 128:hh * 512 + (m + 1) * 128],
                            compare_op=mybir.AluOpType.is_ge,
                            fill=0.0, base=0, pattern=[[1, 128]],
                            channel_multiplier=-1,
                        )
                    return [("both", kj, pt)]

                def emit_PV(si, kj, h, ya, pt, pv_count):
                    nkj = 4 * si + 4
                    c0 = max(kj - 4 * si, 0) * 128
                    _mm(
                        ya[h][:, c0:512],
                        vaug[kj][:, h, :],
                        pt[:, c0:512] if pt.shape[-1] == 512 else pt,
                        start=(kj == 4 * si),
                        stop=(pv_count[h] == nkj - 1),
                        skip_group_check=True,
                        _label=f"PV.si{si}.kj{kj}.h{h}",
                    )
                    pv_count[h] += 1

                def emit_U(si, jq):
                    ti = si * 4 + jq
                    ub_ev = "act" if si == 0 else "dve"
                    for tag, c0, w, ev in (("Ua", 0, 384, "dve"), ("Ub", 384, 384, ub_ev)):
                        up = psU.tile([128, w], F32, tag=tag, name=f"{tag}{si}_{jq}")
                        for h in range(2):
                            _mm(
                                up,
                                yns[si][h][:, jq * 128:(jq + 1) * 128],
                                wga[:, h, c0:c0 + w],
                                start=(h == 0), stop=(h == 1),
                                _label=f"U.si{si}.jq{jq}.{tag}.h{h}",
                            )
                        us = sbU.tile([128, w], BF16, tag=f"us{tag}",
                                      name=f"us{tag}{si}_{jq}")
                        if ev == "dve":
                            nc.vector.tensor_copy(us, up)
                        else:
                            nc.scalar.copy(us, up)
                        nc.sync.dma_start(
                            out=out_d[ti * 128:(ti + 1) * 128, c0:c0 + w], in_=us)

                def norm(si, h, src, q0, q1, alt=False):
                    """yn[si][h][:, q0:q1] = src[:, q0:q1] / l. With alt,
                    use ACT/Pool instead of DVE so two heads' chains run in
                    parallel (src must be SBUF then)."""
                    rr = sbR.tile([1, 512], F32, tag="rr", name=f"rr{si}_{h}_{q0}")
                    rw = rr[:, 0:q1 - q0]
                    nc.vector.reciprocal(rw, src[D:D + 1, q0:q1])
                    rb = sbR.tile([D + 1, 512], F32, tag="rb", name=f"rb{si}_{h}_{q0}")
                    rbw = rb[:, 0:q1 - q0]
                    nc.gpsimd.partition_broadcast(rbw, rw)
                    nc.vector.tensor_mul(
                        yns[si][h][:, q0:q1], src[:, q0:q1], rbw)

                pending_u = []
                for si in (1, 2, 3, 0):
                    ya = [psY.tile([D + 1, 512], F32, name=f"ya{si}_{h}", tag=f"ya{h}")
                          for h in range(2)]
                    # interleave diagonal blocks with full-kj pairs: the PV
                    # accumulation starts at the m=0 diagonal (full width),
                    # and each exp->affine diagonal chain gets a pair round
                    # of PE runway
                    diags = [(kj,) for kj in range(4 * si, 4 * si + 4)]
                    pairs = [(kj, kj + 1) for kj in range(0, 4 * si, 2)]
                    # two diagonal rounds up front so the first pair's S has
                    # psS slack (exp of d0 gets two rounds of runway)
                    rounds = []
                    for ri in range(max(len(diags), len(pairs) + 2)):
                        if ri < len(diags):
                            rounds.append(diags[ri])
                        if 0 <= ri - 2 < len(pairs):
                            rounds.append(pairs[ri - 2])

                    pv_count = [0, 0]

                    def flush(prev_h, h):
                        for item in prev_h:
                            if item[0] == "both":
                                _, kj, pv = item
                                emit_PV(si, kj, h, ya,
                                        pv[:, h * 512:(h + 1) * 512], pv_count)
                            else:
                                kj, pv = item
                                emit_PV(si, kj, h, ya, pv, pv_count)

                    # PV runs two rounds behind S: one round of lag leaves
                    # the PE systematically ~0.5us short of the exp latency
                    pend = [[], []]
                    for ri, kjs in enumerate(rounds):
                        if len(kjs) == 1:
                            out = emit_S(si, kjs, 0)   # both heads inside
                            for h in range(2):
                                pend[h].append(out)
                        else:
                            for h in range(2):
                                pend[h].append(emit_S(si, kjs, h))
                        for h in range(2):
                            if len(pend[h]) > 2:
                                flush(pend[h].pop(0), h)
                        if pending_u and ri >= (1 if si == 0 else 3):
                            emit_U(*pending_u.pop(0))
                            if si == 0 and pending_u and ri >= 2:
                                emit_U(*pending_u.pop(0))
                    # round-major drain with U pops as PE filler
                    for i0, i1 in zip(pend[0], pend[1]):
                        flush(i0, 0)
                        flush(i1, 1)
                        if pending_u:
                            emit_U(*pending_u.pop(0))
                    pend = [[], []]

                    if si != 0:
                        # direct 2-op chain on the PSUM tile (recip then mul)
                        for h in range(2):
                            norm(si, h, ya[h], 0, 512)
                        pending_u.extend((si, jq) for jq in range(4))
                    else:
                        # last superblock: single PSUM read into SBUF, then
                        # chunk normalization + projection per t-block to
                        # pipeline the tail
                        yac = [sbN.tile([D + 1, 512], BF16, tag=f"yac{h}",
                                        name=f"yac0_{h}") for h in range(2)]
                        nc.vector.tensor_copy(yac[0], ya[0])
                        nc.scalar.copy(yac[1], ya[1])
                        for jq in range(4):
                            for h in range(2):
                                norm(0, h, yac[h], jq * 128, (jq + 1) * 128,
                                     alt=(h == 1))
                            if pending_u:
                                emit_U(*pending_u.pop(0))
                            emit_U(0, jq)
                for u in pending_u:
                    emit_U(*u)

    nc.finalize()
    return nc


def _get_nc():
    global _NC_CACHE
    if _NC_CACHE is None:
        _NC_CACHE = _build()
    return _NC_CACHE


def kernel(x, W_attn, b_attn, W_proj, b_proj):
    x = np.asarray(x, dtype=np.float32)
    W_attn = np.asarray(W_attn, dtype=np.float32)
    b_attn = np.asarray(b_attn, dtype=np.float32)
    W_proj = np.asarray(W_proj, dtype=np.float32)
    b_proj = np.asarray(b_proj, dtype=np.float32)

    in_maps = []
    for core in range(8):
        b, hg = core // 4, core % 4
        heads = (2 * hg, 2 * hg + 1)
        # x^T as [128, qr, cc, 512]
        xT = np.ascontiguousarray(
            x[b].T.reshape(CC, 128, NSUP, 512).transpose(1, 2, 0, 3)
        ).astype(NPBF)
        # wqkv [128, cc, 576]: features q0*s | q1*s | k0 | k1 | v0 | v1
        cols, bvals = [], []
        for sec in range(3):
            for h in heads:
                sl = slice(sec * C + h * D, sec * C + (h + 1) * D)
                wc = W_attn[:, sl]
                bb = b_attn[sl]
                if sec == 0:
                    wc = wc * SCALE
                    bb = bb * SCALE
                cols.append(wc)
                bvals.append(bb)
        wqkv = np.ascontiguousarray(
            np.concatenate(cols, axis=1).reshape(CC, 128, 6 * D)
            .transpose(1, 0, 2)).astype(NPBF)
        bcat = np.concatenate(bvals[:4])                  # [384] qk biases
        bqk = np.zeros((128, 3), dtype=np.float32)
        for g in range(3):
            bqk[:, g] = bcat[g * 128:(g + 1) * 128]
        # wga [97, 2, 768]: proj weights per head; row 96 of head 0 carries
        # the folded bias (v-bias @ W_proj for this head pair, plus b_proj
        # exactly once on core 0)
        wga = np.zeros((D + 1, 2, C), dtype=np.float32)
        beff = np.zeros(C, dtype=np.float32)
        for i, h in enumerate(heads):
            wga[0:D, i, :] = W_proj[h * D:(h + 1) * D, :]
            bv = b_attn[2 * C + h * D:2 * C + (h + 1) * D]
            beff += bv @ W_proj[h * D:(h + 1) * D, :]
        if hg == 0:
            beff += b_proj
        wga[D, 0, :] = beff
        in_maps.append({
            "xT": xT, "wqkv": wqkv, "bqk": bqk,
            "wga": wga.astype(NPBF),
        })

    nc = _get_nc()
    kwargs = {}
    if TRACE:
        kwargs = dict(trace=True, trace_cores=[0])
    try:
        res = run_bass_kernel_spmd(nc, in_maps, core_ids=list(range(8)), **kwargs)
    except Exception:
        # transient NRT_EXEC_UNIT_UNRECOVERABLE has been observed on first
        # load; one retry after a pause has always recovered
        import time
        time.sleep(15)
        res = run_bass_kernel_spmd(nc, in_maps, core_ids=list(range(8)), **kwargs)
    global LAST_RESULT
    LAST_RESULT = res
    out = np.zeros((B, T, C), dtype=np.float32)
    for core in range(8):
        out[core // 4] += res.results[core]["out"].astype(np.float32)
    return out

